# revision 2
# baseline (speedup 1.0000x reference)
"""DCN cross-layer kernel for Trainium2 (8 NeuronCores, data-parallel).

Reference computation (L=3 layers):
    x_{l+1} = x0 * (x_l . w_l) + b_l + x_l

Algebraic collapse (same as before): with x_l = x0 * sigma_l + B_l,
    d_l         = x0 . w_l                 (per-row dot, original x0!)
    sigma_{l+1} = sigma_l * (1 + d_l) + beta_l,  beta_l = B_l . w_l (host)
    out         = x0 * sigma_3 + B_3
One streaming pass over x.  Memory-bound.

bf16 data path: x is cast to bf16 on the host and streamed bf16 both
ways (load bf16 x, store bf16 out, upcast f32 on the host after the
gather) — halves HBM traffic vs f32.  Max-normalized error ~5e-3,
comfortably under the 2e-2 gate.

Packed-pair transpose: the bf16 x tile is bitcast to f32 ([128, 512]
"pair" matrix), PE-transposed via identity, and copied PSUM->SBUF once
per tile ([128, 512] f32 — half the f32 copy volume of a plain f32
pipeline).  The transposed tile is bitcast back to bf16, giving
xts[p, c, b, s] = x[b, 2*(c*128+p)+s]; the per-row dots accumulate over
8 (chunk, phase) matmuls with stride-2 lhsT APs against a host-packed
W^T whose layout matches.

Engine split per [128, 1024] tile:
  PE   : 4 transposes (f32 pairs) + 8 dot matmuls (K=128, M=128, N=3)
  DVE  : 1 PSUM->SBUF copy [128, 512] f32 + 3 tiny recurrence ops
  ACT  : final scaled copy out = x0 * sigma_3 (bf16)
  DMA  : 256 KiB load + 256 KiB store (bf16)
"""

import numpy as np
import ml_dtypes

BF16 = ml_dtypes.bfloat16

N_CORES = 8
B, D = 8192, 1024
L = 3
B_SH = B // N_CORES  # 1024 rows per core
P = 128
N_TILES = B_SH // P  # 8 tiles of [128, 1024] per core
N_PK = D // 2        # 512 packed bf16 pairs per row
N_PCH = N_PK // P    # 4 packed chunks per tile

LAST_RESULTS = None  # BassKernelResults of the most recent run (for test.py)


def _build_program(betas, has_b3):
    import concourse.bacc as bacc
    import concourse.tile as tile
    from concourse import mybir
    from concourse.masks import make_identity

    f32 = mybir.dt.float32
    bf16 = mybir.dt.bfloat16
    mult = mybir.AluOpType.mult
    add = mybir.AluOpType.add

    nc = bacc.Bacc("TRN2", target_bir_lowering=False, debug=False,
                   num_devices=N_CORES)

    x_d = nc.dram_tensor("x", [B_SH, D], bf16, kind="ExternalInput").ap()
    wt_d = nc.dram_tensor("wt", [P, N_PCH * 2 * L], bf16,
                          kind="ExternalInput").ap()
    out_d = nc.dram_tensor("out", [B_SH, D], bf16, kind="ExternalOutput").ap()
    b3_d = None
    if has_b3:
        b3_d = nc.dram_tensor("b3b", [P, D], bf16, kind="ExternalInput").ap()

    with tile.TileContext(nc) as tc:
        with (
            tc.tile_pool(name="const", bufs=1) as const_pool,
            # full-depth SBUF pools: no slot reuse -> no false waits
            # chaining compute behind stores
            tc.tile_pool(name="xin", bufs=N_TILES) as xin,
            tc.tile_pool(name="xtp", bufs=N_TILES) as xtp,
            tc.tile_pool(name="outp", bufs=N_TILES) as outp,
            tc.tile_pool(name="small", bufs=N_TILES) as small,
            tc.tile_pool(name="ptp", bufs=4, space="PSUM") as ptp,
            tc.tile_pool(name="pd", bufs=3, space="PSUM") as pd,
        ):
            # identity on gpsimd (Pool engine is otherwise idle)
            ident = const_pool.tile([P, P], f32, tag="ident")
            make_identity(nc, ident[:])
            if has_b3:
                b3 = const_pool.tile([P, D], bf16, tag="b3")
                nc.sync.dma_start(b3[:], b3_d[:])

            # issue every load before any store: all loads precede stores
            # in the SP sequencer order, so a compute op waiting for its
            # load never transitively waits on store completions
            xts_all = []
            wt_sb = None
            for t in range(N_TILES):
                xt = xin.tile([P, D], bf16, tag="xt")
                nc.sync.dma_start(xt[:], x_d[t * P:(t + 1) * P, :])
                xts_all.append(xt)
                if t == 0:
                    # tiny W^T load goes second: x tile 0 transfers first
                    # so PE can start sooner
                    wt_sb = const_pool.tile([P, N_PCH * 2 * L], bf16,
                                            tag="wt")
                    nc.sync.dma_start(wt_sb[:], wt_d[:])

            wv = wt_sb[:].rearrange("p (c s l) -> p c s l",
                                    c=N_PCH, s=2, l=L)

            for t in range(N_TILES):
                xt = xts_all[t]
                xpk = xt[:].bitcast(f32)  # [128, 512] packed pairs

                # transpose the packed-pair matrix: tp[p, c*128+b] =
                # xpk[b, c*128+p]
                tp = ptp.tile([P, N_PK], f32, tag="tp")
                for c in range(N_PCH):
                    nc.tensor.transpose(
                        tp[:, c * P:(c + 1) * P],
                        xpk[:, c * P:(c + 1) * P],
                        ident[:])

                xts = xtp.tile([P, N_PK], f32, tag="xts")
                nc.vector.tensor_copy(xts[:], tp[:])
                # bf16 view: xv[p, c, b, s] = x[b, 2*(c*128+p)+s]
                xv = xts[:].bitcast(bf16).rearrange(
                    "p (c b s) -> p c b s", c=N_PCH, b=P, s=2)

                # d[b, l] = sum_d x0[b, d] W[l, d], accumulated over the
                # 8 (chunk, phase) pieces
                dps = pd.tile([P, L], f32, tag="dps")
                dcol = dps[:, 0:L]
                k = 0
                for c in range(N_PCH):
                    for s in range(2):
                        nc.tensor.matmul(
                            dcol,
                            xv[:, c, :, s],
                            wv[:, c, s, :],
                            start=(k == 0),
                            stop=(k == 2 * N_PCH - 1))
                        k += 1

                # sigma recurrence: sigma_{l+1} = sigma_l*(1+d_l) + beta_l
                sig = small.tile([P, 1], f32, tag="sig1")
                nc.vector.tensor_scalar_add(sig[:], dcol[:, 0:1],
                                            1.0 + betas[0])
                for l in (1, 2):
                    nsig = small.tile([P, 1], f32, tag=f"sig{l + 1}")
                    nc.vector.scalar_tensor_tensor(
                        out=nsig[:], in0=dcol[:, l:l + 1], scalar=1.0,
                        in1=sig[:], op0=add, op1=mult)
                    if betas[l] != 0.0:
                        nc.vector.tensor_scalar_add(nsig[:], nsig[:],
                                                    float(betas[l]))
                    sig = nsig

                # out = x0 * sigma_3 (+ B3) — scaled copy on ACT
                ot = outp.tile([P, D], bf16, tag="ot")
                nc.scalar.mul(ot[:], xt[:], sig[:])
                if has_b3:
                    nc.vector.tensor_add(ot[:], ot[:], b3[:])
                nc.sync.dma_start(out_d[t * P:(t + 1) * P, :], ot[:])

    nc.compile()
    return nc


def predict_time_ns(trace_path=None):
    """Single-core timeline-sim of the kernel program (cost-model time in
    ns).  SPMD data-parallel with no collectives, so per-core time ==
    kernel time.  Optionally writes a perfetto trace."""
    from trails.perfetto import LazyPerfetto
    for _m in ("enable_explicit_ordering", "reserve_process_order",
               "add_counter", "add_flow", "add_instant"):
        if not hasattr(LazyPerfetto, _m):
            setattr(LazyPerfetto, _m, lambda self, *a, **k: None)
    from concourse.timeline_sim import TimelineSim

    nc = _build_program([0.0, 0.0, 0.0], False)
    tlsim = TimelineSim(nc, trace=trace_path is not None)
    tlsim.simulate()
    if trace_path is not None and tlsim.perfetto is not None:
        tlsim.perfetto.save(trace_path)
    return tlsim.time


def _pack_wt(W):
    """wt[p, (c, s, l)] = W[l, 2*(c*128+p)+s] as bf16 [128, 24]."""
    Wt = np.ascontiguousarray(W.T)                 # [D, L] = [(j s), l]
    Wp = Wt.reshape(N_PK, 2, L)                    # [j, s, l]
    Wp = Wp.reshape(N_PCH, P, 2, L)                # [c, p, s, l]
    Wp = Wp.transpose(1, 0, 2, 3)                  # [p, c, s, l]
    return np.ascontiguousarray(Wp.reshape(P, N_PCH * 2 * L)).astype(BF16)


def kernel(x, W, b):
    global LAST_RESULTS
    from concourse.bass_utils import run_bass_kernel_spmd

    x = np.asarray(x, dtype=np.float32)
    W = np.asarray(W, dtype=np.float32)
    b = np.asarray(b, dtype=np.float32)

    # Host precompute: beta_l = (sum_{j<l} b_j) . w_l  and B_3 = sum_l b_l.
    Bl = np.zeros(D, dtype=np.float64)
    betas = []
    for l in range(L):
        betas.append(float(Bl @ W[l].astype(np.float64)))
        Bl = Bl + b[l].astype(np.float64)
    B3 = Bl.astype(np.float32)
    has_b3 = bool(np.any(B3))

    nc = _build_program(betas, has_b3)

    xb = np.ascontiguousarray(x).astype(BF16)      # bf16 stream-in copy
    wt_host = _pack_wt(W)
    in_maps = []
    for i in range(N_CORES):
        m = {"x": xb[i * B_SH:(i + 1) * B_SH], "wt": wt_host}
        if has_b3:
            m["b3b"] = np.ascontiguousarray(
                np.broadcast_to(B3.astype(BF16), (P, D)))
        in_maps.append(m)

    res = run_bass_kernel_spmd(nc, in_maps, core_ids=list(range(N_CORES)))
    LAST_RESULTS = res
    out = np.concatenate([res.results[i]["out"] for i in range(N_CORES)],
                         axis=0)
    return out.astype(np.float32)


# revision 19
# speedup vs baseline: 1.3337x; 1.3337x over previous
"""DCN cross-layer kernel for Trainium2 (8 NeuronCores, data-parallel).

Reference computation (L=3 layers):
    x_{l+1} = x0 * (x_l . w_l) + b_l + x_l

Algebraic collapse (same as before): with x_l = x0 * sigma_l + B_l,
    d_l         = x0 . w_l                 (per-row dot, original x0!)
    sigma_{l+1} = sigma_l * (1 + d_l) + beta_l,  beta_l = B_l . w_l (host)
    out         = x0 * sigma_3 + B_3
One streaming pass over x.  Memory-bound.

bf16 data path: x is cast to bf16 on the host and streamed bf16 both
ways (load bf16 x, store bf16 out, upcast f32 on the host after the
gather) — halves HBM traffic vs f32.  Max-normalized error ~5e-3,
comfortably under the 2e-2 gate.

Packed-pair transpose: the bf16 x tile is bitcast to f32 ([128, 512]
"pair" matrix), PE-transposed via identity, and copied PSUM->SBUF once
per tile ([128, 512] f32 — half the f32 copy volume of a plain f32
pipeline).  The transposed tile is bitcast back to bf16, giving
xts[p, c, b, s] = x[b, 2*(c*128+p)+s]; the per-row dots accumulate over
8 (chunk, phase) matmuls with stride-2 lhsT APs against a host-packed
W^T whose layout matches.

Engine split per [128, 1024] tile:
  PE   : 4 transposes (f32 pairs) + 8 dot matmuls (K=128, M=128, N=3)
  DVE  : 1 PSUM->SBUF copy [128, 512] f32 + 3 tiny recurrence ops
  ACT  : final scaled copy out = x0 * sigma_3 (bf16)
  DMA  : 256 KiB load + 256 KiB store (bf16)
"""

import numpy as np
import ml_dtypes

BF16 = ml_dtypes.bfloat16

N_CORES = 8
B, D = 8192, 1024
L = 3
B_SH = B // N_CORES  # 1024 rows per core
P = 128
N_TILES = B_SH // P  # 8 tiles of [128, 1024] per core
N_PK = D // 2        # 512 packed bf16 pairs per row
N_PCH = N_PK // P    # 4 packed chunks per tile
WT_COLS = N_PCH * 2 * L  # 24 extra cols on tile 0 carrying packed W^T

LAST_RESULTS = None  # BassKernelResults of the most recent run (for test.py)


def _build_program(betas, has_b3):
    import concourse.bacc as bacc
    import concourse.tile as tile
    from concourse import mybir
    from concourse.masks import make_identity

    f32 = mybir.dt.float32
    bf16 = mybir.dt.bfloat16
    mult = mybir.AluOpType.mult
    add = mybir.AluOpType.add

    nc = bacc.Bacc("TRN2", target_bir_lowering=False, debug=False,
                   num_devices=N_CORES)

    # tile 0's rows carry W^T packed into 24 extra columns: one DMA loads
    # both, so the tiny W transfer never bubbles the HWDGE/DMA stream
    x0w_d = nc.dram_tensor("x0w", [P, D + WT_COLS], bf16,
                           kind="ExternalInput").ap()
    xr_d = nc.dram_tensor("xr", [B_SH - P, D], bf16,
                          kind="ExternalInput").ap()
    out_d = nc.dram_tensor("out", [B_SH, D], bf16, kind="ExternalOutput").ap()
    b3_d = None
    if has_b3:
        b3_d = nc.dram_tensor("b3b", [P, D], bf16, kind="ExternalInput").ap()

    with tile.TileContext(nc) as tc:
        with (
            tc.tile_pool(name="const", bufs=1) as const_pool,
            # full-depth SBUF pools: no slot reuse -> no false waits
            # chaining compute behind stores
            tc.tile_pool(name="xin", bufs=N_TILES) as xin,
            tc.tile_pool(name="xtp", bufs=N_TILES) as xtp,
            tc.tile_pool(name="outp", bufs=N_TILES) as outp,
            tc.tile_pool(name="small", bufs=N_TILES) as small,
            tc.tile_pool(name="ptp", bufs=3, space="PSUM") as ptp,
            tc.tile_pool(name="pd", bufs=3, space="PSUM") as pd,
            tc.tile_pool(name="pwarm", bufs=1, space="PSUM") as pwarm,
        ):
            # identity on gpsimd (Pool engine is otherwise idle early)
            ident = const_pool.tile([P, P], f32, tag="ident")
            make_identity(nc, ident[:])
            if has_b3:
                b3 = const_pool.tile([P, D], bf16, tag="b3")
                nc.scalar.dma_start(b3[:], b3_d[:])

            # PE warmup: the cost model ramps PE from 1.54ns/cycle (cold)
            # through 0.83 (100ns+ busy) to 0.42 (3us+ continuous busy).
            # Junk matmuls on a DVE-zeroed tile keep PE continuously busy
            # from ~0.3us (before the identity even exists), so the real
            # tile-0 transposes at ~3.6us run at full speed.
            junk = const_pool.tile([P, P], bf16, tag="junk")
            nc.vector.memset(junk[:], 0.0)
            jview = junk[:].bitcast(f32)  # [128, 64] f32 view
            warm = pwarm.tile([P, 512], f32, tag="warm")
            # one long f32 matmul (4 cyc/row, 448 moving rows) burns
            # ~2.8us of ramp at cold pstate, then short ones bridge
            # until the first x tile arrives (~3.6us)
            nc.tensor.matmul(warm[0:64, 0:448], jview,
                             jview[:, 0:1].to_broadcast((P, 448)),
                             start=True, stop=True)
            for _ in range(5):
                nc.tensor.matmul(warm[0:P, 0:P], junk[:], junk[:],
                                 start=True, stop=True)

            # issue every load before any store: all loads precede stores
            # in the SP sequencer order, so a compute op waiting for its
            # load never transitively waits on store completions
            xts_all = []
            for t in range(N_TILES):
                if t == 0:
                    xt = xin.tile([P, D + WT_COLS], bf16, tag="xt0")
                    nc.sync.dma_start(xt[:], x0w_d[:])
                else:
                    xt = xin.tile([P, D], bf16, tag="xt")
                    nc.sync.dma_start(
                        xt[:], xr_d[(t - 1) * P:t * P, :])
                xts_all.append(xt)

            wv = xts_all[0][:, D:D + WT_COLS].rearrange(
                "p (c s l) -> p c s l", c=N_PCH, s=2, l=L)

            for t in range(N_TILES):
                xt = xts_all[t][:, 0:D]
                xpk = xt.bitcast(f32)  # [128, 512] packed pairs

                # transpose the packed-pair matrix: tp[p, c*128+b] =
                # xpk[b, c*128+p]
                tp = ptp.tile([P, N_PK], f32, tag="tp")
                for c in range(N_PCH):
                    nc.tensor.transpose(
                        tp[:, c * P:(c + 1) * P],
                        xpk[:, c * P:(c + 1) * P],
                        ident[:])

                xts = xtp.tile([P, N_PK], f32, tag="xts")
                # PSUM->SBUF on ACT (570ns) — cheaper than DVE for f32,
                # and keeps DVE free for the 2x-rate bf16 output multiply
                nc.scalar.copy(xts[:], tp[:])
                # bf16 view: xv[p, c, b, s] = x[b, 2*(c*128+p)+s]
                xv = xts[:].bitcast(bf16).rearrange(
                    "p (c b s) -> p c b s", c=N_PCH, b=P, s=2)

                # d[b, l] = sum_d x0[b, d] W[l, d], accumulated over the
                # 8 (chunk, phase) pieces
                dps = pd.tile([P, L], f32, tag="dps")
                dcol = dps[:, 0:L]
                k = 0
                for c in range(N_PCH):
                    for s in range(2):
                        nc.tensor.matmul(
                            dcol,
                            xv[:, c, :, s],
                            wv[:, c, s, :],
                            start=(k == 0),
                            stop=(k == 2 * N_PCH - 1))
                        k += 1

                # sigma recurrence on DVE: same engine as the output
                # multiply, so no cross-engine semaphore hops in the
                # per-tile critical chain
                sig = small.tile([P, 1], f32, tag="sig1")
                nc.vector.tensor_scalar_add(sig[:], dcol[:, 0:1],
                                            1.0 + betas[0])
                for l in (1, 2):
                    nsig = small.tile([P, 1], f32, tag=f"sig{l + 1}")
                    nc.vector.scalar_tensor_tensor(
                        out=nsig[:], in0=dcol[:, l:l + 1], scalar=1.0,
                        in1=sig[:], op0=add, op1=mult)
                    if betas[l] != 0.0:
                        nc.vector.tensor_scalar_add(nsig[:], nsig[:],
                                                    float(betas[l]))
                    sig = nsig

                # out = x0 * sigma_3 (+ B3) — bf16 multiply on DVE runs in
                # 2x perf mode (654ns), vs 1038ns for the same op on ACT
                ot = outp.tile([P, D], bf16, tag="ot")
                nc.vector.tensor_scalar_mul(ot[:], xt, sig[:])
                if has_b3:
                    nc.vector.tensor_add(ot[:], ot[:], b3[:])
                nc.sync.dma_start(out_d[t * P:(t + 1) * P, :], ot[:])

    nc.compile()
    return nc


def predict_time_ns(trace_path=None):
    """Single-core timeline-sim of the kernel program (cost-model time in
    ns).  SPMD data-parallel with no collectives, so per-core time ==
    kernel time.  Optionally writes a perfetto trace."""
    from trails.perfetto import LazyPerfetto
    for _m in ("enable_explicit_ordering", "reserve_process_order",
               "add_counter", "add_flow", "add_instant"):
        if not hasattr(LazyPerfetto, _m):
            setattr(LazyPerfetto, _m, lambda self, *a, **k: None)
    from concourse.timeline_sim import TimelineSim

    nc = _build_program([0.0, 0.0, 0.0], False)
    tlsim = TimelineSim(nc, trace=trace_path is not None)
    tlsim.simulate()
    if trace_path is not None and tlsim.perfetto is not None:
        tlsim.perfetto.save(trace_path)
    return tlsim.time


def _pack_wt(W):
    """wt[p, (c, s, l)] = W[l, 2*(c*128+p)+s] as bf16 [128, 24]."""
    Wt = np.ascontiguousarray(W.T)                 # [D, L] = [(j s), l]
    Wp = Wt.reshape(N_PK, 2, L)                    # [j, s, l]
    Wp = Wp.reshape(N_PCH, P, 2, L)                # [c, p, s, l]
    Wp = Wp.transpose(1, 0, 2, 3)                  # [p, c, s, l]
    return np.ascontiguousarray(Wp.reshape(P, N_PCH * 2 * L)).astype(BF16)


def kernel(x, W, b):
    global LAST_RESULTS
    from concourse.bass_utils import run_bass_kernel_spmd

    x = np.asarray(x, dtype=np.float32)
    W = np.asarray(W, dtype=np.float32)
    b = np.asarray(b, dtype=np.float32)

    # Host precompute: beta_l = (sum_{j<l} b_j) . w_l  and B_3 = sum_l b_l.
    Bl = np.zeros(D, dtype=np.float64)
    betas = []
    for l in range(L):
        betas.append(float(Bl @ W[l].astype(np.float64)))
        Bl = Bl + b[l].astype(np.float64)
    B3 = Bl.astype(np.float32)
    has_b3 = bool(np.any(B3))

    nc = _build_program(betas, has_b3)

    xb = np.ascontiguousarray(x).astype(BF16)      # bf16 stream-in copy
    wt_host = _pack_wt(W)
    in_maps = []
    for i in range(N_CORES):
        shard = xb[i * B_SH:(i + 1) * B_SH]
        m = {
            # tile 0's rows carry packed W^T in 24 extra columns
            "x0w": np.ascontiguousarray(
                np.concatenate([shard[:P], wt_host], axis=1)),
            "xr": np.ascontiguousarray(shard[P:]),
        }
        if has_b3:
            m["b3b"] = np.ascontiguousarray(
                np.broadcast_to(B3.astype(BF16), (P, D)))
        in_maps.append(m)

    res = run_bass_kernel_spmd(nc, in_maps, core_ids=list(range(N_CORES)))
    LAST_RESULTS = res
    out = np.concatenate([res.results[i]["out"] for i in range(N_CORES)],
                         axis=0)
    return out.astype(np.float32)
